# revision 2
# baseline (speedup 1.0000x reference)
"""Trainium2 Bass kernel: 7x7 valid cross-correlation + bias on a 4096x4096 f32 image.

Formulation: banded matmul on the TensorEngine.
  out[r, c] = sum_{di,dj} w[di,dj] * x[r+di, c+dj]
For an output row-strip of M=122 rows starting at r0, using K=128 input rows:
  out[r0+m, c] = sum_k A_dj[k, m] * x[r0+k, c+dj]   summed over dj=0..6
where A_dj[k, m] = w[k-m, dj] for 0 <= k-m < 7 (a banded [128, 122] matrix,
precomputed on host from the 49 kernel weights). The 7 dj-terms accumulate
into one PSUM bank via shifted column slices of the same SBUF rhs tile.

Matmuls run in bf16 (1 col/cycle on the PE vs 4 for fp32; fp32 PSUM accum);
the output is written back as bf16 and upcast on the host. Measured rel-err
vs the fp32 reference is ~4e-3, inside the 2e-2 gate.

Schedule (v2, trace-driven):
  - exec_time is measured from the first USER instruction to the end of the
    NEFF epilogue, so the framework preamble is free but everything after
    the first memset counts.
  - HAM warmup: the PE clock sits at 1.2GHz until ~3.4us of sustained
    matmul activity. A burst of dummy matmuls on a memset scratch tile runs
    during the input-DMA spin-up window so all real matmuls issue at 2.4GHz
    (216ns per N=512 bf16 matmul, measured).
  - Input DMAs all on the Sync HWDGE queue, in-order: band matrices first
    (224KB), then strip chunks [1,2,4,6,7,7,7]; the first real matmul is
    gated only by bands+strip0 (~3.5us incl. ~2.5us queue spin-up). DMA
    delivery (~290GB/s observed) outruns PE consumption (~88GB/s) 3x.
  - PSUM groups of [2,2,4,4,4,4,4,4,2,2,1,1] strips; dj is the outer loop
    within a group so matmuls sharing a stationary band run back-to-back;
    bank reuse distance >= 2 groups (8-bank pool).
  - PSUM evacuation alternates Vector (tensor_tensor add) and Scalar
    (activation Identity with bias) so two drains run concurrently.
  - Stores: one SWDGE DMA per group (packets spread over all 16 SDMA
    engines; ~900ns issue each on GpSimd). The final 1-strip groups drain
    split across Vector+Scalar and store via HWDGE on the idle Sync queue
    to skip the SWDGE issue cost and ring queueing at the tail.

Sharding: output columns split across the 8 cores (512 cols/core); each
core processes all 4090 output rows. Kernel + bias replicated.
"""

import numpy as np

H, W = 4096, 4096
KH, KW = 7, 7
OH, OW = H - KH + 1, W - KW + 1  # 4090, 4090
N_CORES = 8
CW = 512               # output columns per core
IW = CW + KW - 1       # input columns per core (518)
STRIP = 122            # output rows per strip (K = STRIP + KH - 1 = 128)
MB = 128               # stationary block columns (M padded 122 -> 128)
N_STRIPS = (OH + STRIP - 1) // STRIP  # 34 (last strip M=64, K=70)

GROUPS = [2, 2, 4, 4, 4, 4, 4, 4, 2, 2, 1, 1]   # strips per PSUM group
IN_CHUNKS = [1, 2, 4, 6, 7, 7, 7]               # strips per input DMA
N_WARM = 8                                       # dummy matmuls for HAM warmup

assert sum(GROUPS) == N_STRIPS and sum(IN_CHUNKS) == N_STRIPS

_cache = {}


def _build_nc():
    import concourse.bacc as bacc
    import concourse.mybir as mybir
    from concourse.tile import TileContext

    f32 = mybir.dt.float32
    bf16 = mybir.dt.bfloat16

    nc = bacc.Bacc("TRN2", target_bir_lowering=False, debug=False)
    xs = nc.dram_tensor("xs", [128, N_STRIPS * IW], bf16, kind="ExternalInput")
    bands = nc.dram_tensor("bands", [128, KW * MB], bf16, kind="ExternalInput")
    biasv = nc.dram_tensor("biasv", [128, 1], f32, kind="ExternalInput")
    # Packed output: out[m, s*CW + c] = out_full[122*s + m, c]; host unpacks.
    out = nc.dram_tensor("out", [STRIP, N_STRIPS * CW], bf16, kind="ExternalOutput")

    with TileContext(nc) as tc:
        with (
            tc.tile_pool(name="const", bufs=1) as cpool,
            tc.tile_pool(name="rhs", bufs=7) as rpool,
            tc.tile_pool(name="obuf", bufs=6) as opool,
            tc.tile_pool(name="psum", bufs=8, space="PSUM") as ppool,
        ):
            # Warmup scratch + bias landing pad first: tiny ops, no DMA deps.
            warm_t = cpool.tile([128, 640], bf16)
            nc.vector.memset(warm_t[:, :], 0.0)
            bias1_t = cpool.tile([128, 1], f32)
            nc.scalar.dma_start(out=bias1_t[:, :], in_=biasv[:, :])

            # All input loads on the Sync HWDGE queue, in-order: bands first
            # so the first LDWEIGHTS unblocks as early as possible.
            band_t = cpool.tile([128, KW * MB], bf16)
            nc.sync.dma_start(out=band_t[:, :], in_=bands[:, :])
            strip_tile = {}
            s0 = 0
            for n in IN_CHUNKS:
                xt = rpool.tile([128, max(IN_CHUNKS) * IW], bf16, tag="rhs")
                nc.sync.dma_start(
                    out=xt[:, : n * IW], in_=xs[:, s0 * IW : (s0 + n) * IW]
                )
                for j in range(n):
                    strip_tile[s0 + j] = (xt, j * IW)
                s0 += n

            # HAM warmup: ~3.4us of dummy matmuls (cold: 427ns each) on the
            # memset scratch while the first input chunks are in flight, so
            # the PE clock is at 2.4GHz when real matmuls start.
            warm_ps = ppool.tile([128, CW], f32, name="ps", tag="ps")
            for _ in range(N_WARM):
                nc.tensor.matmul(
                    warm_ps[:, :],
                    warm_t[:, :128],
                    warm_t[:, 128:640],
                    start=True,
                    stop=True,
                )
            # broadcast bias to [128, CW] on-chip for the Vector drains
            bias_t = cpool.tile([128, CW], f32)
            nc.vector.tensor_scalar_add(
                bias_t[:, :], warm_t[:, :CW], bias1_t[:, :1]
            )

            strips_done = 0
            n_groups = len(GROUPS)
            for gi, n in enumerate(GROUPS):
                s0 = strips_done
                strips = list(range(s0, s0 + n))
                strips_done += n
                dims = []
                for s in strips:
                    r0 = s * STRIP
                    dims.append((r0, min(STRIP, OH - r0), min(128, H - r0)))
                ps_ts = [
                    ppool.tile([128, CW], f32, name="ps", tag="ps") for _ in strips
                ]
                for dj in range(KW):
                    lhsT = band_t[:, dj * MB : dj * MB + MB]
                    for j, (r0, M, K) in enumerate(dims):
                        sxt, soff = strip_tile[strips[j]]
                        nc.tensor.matmul(
                            ps_ts[j][:, :],
                            lhsT[:K, :],
                            sxt[:K, soff + dj : soff + dj + CW],
                            start=(dj == 0),
                            stop=(dj == KW - 1),
                        )
                ot = opool.tile([128, max(GROUPS) * CW], bf16, tag="ot")
                last2 = gi >= n_groups - 2
                for j, (r0, M, K) in enumerate(dims):
                    if last2:
                        # final 1-strip groups: split the drain across both
                        # engines so the store unblocks ~2x sooner
                        h = CW // 2
                        nc.vector.tensor_tensor(
                            ot[:M, j * CW : j * CW + h],
                            ps_ts[j][:M, :h],
                            bias_t[:M, :h],
                            mybir.AluOpType.add,
                        )
                        nc.scalar.activation(
                            ot[:M, j * CW + h : (j + 1) * CW],
                            ps_ts[j][:M, h:],
                            mybir.ActivationFunctionType.Identity,
                            bias=bias1_t[:M, :1],
                        )
                    elif j % 2 == 0:
                        nc.vector.tensor_tensor(
                            ot[:M, j * CW : (j + 1) * CW],
                            ps_ts[j][:M, :],
                            bias_t[:M, :],
                            mybir.AluOpType.add,
                        )
                    else:
                        nc.scalar.activation(
                            ot[:M, j * CW : (j + 1) * CW],
                            ps_ts[j][:M, :],
                            mybir.ActivationFunctionType.Identity,
                            bias=bias1_t[:M, :1],
                        )
                if last2:
                    # tail: HWDGE on the (now idle) Sync queue — cheap issue,
                    # empty ring, no SWDGE descriptor-generation serialization
                    nc.sync.dma_start(
                        out=out[:, s0 * CW : (s0 + n) * CW],
                        in_=ot[:STRIP, : n * CW],
                    )
                else:
                    nc.gpsimd.dma_start(
                        out=out[:, s0 * CW : (s0 + n) * CW],
                        in_=ot[:STRIP, : n * CW],
                    )

    nc.finalize()
    return nc


def _get_nc():
    if "nc" not in _cache:
        _cache["nc"] = _build_nc()
    return _cache["nc"]


def _build_bands(weight: np.ndarray) -> np.ndarray:
    """bands[k, dj*MB + m] = weight[k - m, dj] for 0 <= k-m < KH, m < STRIP."""
    w = np.asarray(weight, np.float32)
    bands = np.zeros((128, KW * MB), np.float32)
    m = np.arange(STRIP)
    for dj in range(KW):
        for di in range(KH):
            bands[m + di, dj * MB + m] = w[di, dj]
    return bands


def _prepare_in_maps(x, weight, bias):
    import ml_dtypes

    bf16 = ml_dtypes.bfloat16
    xb = np.ascontiguousarray(x, np.float32).astype(bf16)
    bands = _build_bands(weight).astype(bf16)
    bias_tile = np.full((128, 1), np.float32(np.asarray(bias).reshape(-1)[0]))

    # xs_packed[k, s, c] = x[122*s + k, c0 + c], zero beyond image edges.
    k_idx = np.arange(128)[:, None]
    s_idx = np.arange(N_STRIPS)[None, :]
    rows = k_idx + STRIP * s_idx  # [128, N_STRIPS]
    row_ok = rows < H
    rows_c = np.minimum(rows, H - 1)

    in_maps = []
    for c in range(N_CORES):
        c0 = c * CW
        avail = min(IW, W - c0)
        xsl = np.zeros((H, IW), bf16)
        xsl[:, :avail] = xb[:, c0 : c0 + avail]
        xs = xsl[rows_c, :]  # [128, N_STRIPS, IW]
        xs[~row_ok] = 0
        xs = np.ascontiguousarray(xs.reshape(128, N_STRIPS * IW))
        in_maps.append({"xs": xs, "bands": bands, "biasv": bias_tile})
    return in_maps


def _gather_out(per_core_outs) -> np.ndarray:
    out = np.empty((OH, OW), np.float32)
    for c in range(N_CORES):
        c0 = c * CW
        take = min(CW, OW - c0)
        po = per_core_outs[c]["out"].astype(np.float32).reshape(STRIP, N_STRIPS, CW)
        full = po.transpose(1, 0, 2).reshape(N_STRIPS * STRIP, CW)
        out[:, c0 : c0 + take] = full[:OH, :take]
    return out


def kernel(x: np.ndarray, weight: np.ndarray, bias: np.ndarray) -> np.ndarray:
    from concourse import bass_utils

    nc = _get_nc()
    in_maps = _prepare_in_maps(x, weight, bias)
    res = bass_utils.run_bass_kernel_spmd(nc, in_maps, list(range(N_CORES)))
    _cache["last_results"] = res
    return _gather_out(res.results)
